# revision 4
# baseline (speedup 1.0000x reference)
"""LFD all-pairs distance kernel for 8 Trainium2 NeuronCores.

Strategy (data-parallel over tgt batch axis m, per sharding hint):
  - Each of the 8 cores owns 16 of the 128 tgt rows (1600 tgt descriptors).
  - The pairwise cost D[t, s] = sum_k w_k * q8_table[idxS[s,k], idxT[t,k]]
    (s = 400 src descriptors, t = 1600 tgt descriptors per core,
     k = 47 coefficient slots: 35 art + 10 fd(w=2) + cir(w=2) + ecc)
    is computed as fp8 one-hot contractions on TensorE with DoubleRow
    (256-row contraction per pass), accumulating in one PSUM tile per
    128-target output tile.  Two complementary factorizations share the
    accumulation; the k-slots are split between them to balance DMA
    bytes against PE passes:
      * tgt-side (per-tile row compression):
            D += B^T @ Rt,  Rt[(k,c), s] = q8[idxS[s,k], c] - 128,
            B[(k,c), t] = w_k if idxT[t,k] == c
        rows = per-tile unique (k, idxT) pairs; costs ~528B DMA/row-use.
      * src-side (row set fixed by the src indices, shared by all tiles
        AND all cores):
            D += Rs^T(t-part) with Bs streamed:
            Rs[(k,c), t] = q8[c, idxT[t,k]] - 128  (stationary, per tile)
            Bs[(k,c), s] = w_k if idxS[s,k] == c   (resident, loaded once)
        rows = global unique (k, idxS) pairs; costs ~128B DMA/row-use.
  - Everything ships as fp8_e4m3 of (table value - 128); |x| <= 128 so no
    TRN e4m3 clipping at 240, rounding err <= 4/el washes out over the
    47-term sums (measured end-to-end rel err ~2e-3 vs 2e-2 gate).  The
    constant 128*sum(w_k) = 7424 is added back on host.
  - Host does index re-encoding (one-hot/gather layouts, gathers of the
    pre-converted fp8 table bytes) + final alignment min-reduction.
"""

import numpy as np
import ml_dtypes

N_SRC = 4
M_TGT = 128
NCORES = 8
MLOC = M_TGT // NCORES      # 16 tgt rows per core
S = N_SRC * 100             # 400 src descriptors
TLOC = MLOC * 100           # 1600 tgt descriptors per core
TILE_T = 128
NT = (TLOC + TILE_T - 1) // TILE_T   # 13 t tiles
K = 47                      # coefficient slots
W_K = np.array([1.0] * 35 + [2.0] * 10 + [2.0, 1.0], np.float32)
SHIFT = 128.0
BASE = SHIFT * float(W_K.sum())      # 7424: added back on host
NS_K = 24                   # number of k-slots handled by the src-side
DR = 256                    # DoubleRow contraction rows per pass

_CACHE = {}


def _install_tile_patch():
    import concourse.mybir as mybir
    from concourse import tile as _tile_mod
    from concourse.vector_clock import ScopedClock as _ScopedClock

    if getattr(_tile_mod.TileContext, "_drain_split_patched", False):
        return

    def _drain_and_barrier(self, tick_clock, wait_clock):
        # walrus's setupSyncWait rejects instructions with many embedded
        # waits; spread the exit-drain's wait set over extra SP nops.
        drain_inst = self.nc.sync.drain()
        wait_clock.add_sem_waits(
            drain_inst.ins,
            _ScopedClock({None: tick_clock.global_clock}))
        si = drain_inst.ins.sync_info
        waits = list(si.on_wait or [])
        if len(waits) > 1:
            si.on_wait = waits[:1]
            for j in range(1, len(waits)):
                nop = self.nc.sync.nop()
                nop.ins.sync_info = mybir.SyncInfo(
                    on_wait=[waits[j]], on_update=[])
        self.nc.all_engine_barrier()
        assert self.sems is not None
        popped = self.nc._tile_sem_poison_stack.pop()
        assert popped is self._sem_poison
        self.nc.clear_and_free_semaphores(
            list(self.sems.allocated().values()))
        self.nc.all_engine_barrier()

    _tile_mod.TileContext._drain_and_barrier = _drain_and_barrier
    _tile_mod.TileContext._drain_split_patched = True


def _build_nc(cs, ct_list, out_slots):
    """cs: src-side DoubleRow pass count (same for every tile/core);
    ct_list: tgt-side DR pass count per tile position; out_slots: output
    slot per position."""
    import concourse.bass as bass
    import concourse.mybir as mybir
    from concourse.tile import TileContext

    _install_tile_patch()

    nc = bass.Bass()
    dr_mode = mybir.MatmulPerfMode.DoubleRow

    rs_ds = [nc.dram_tensor(f"rs{i}", [128, cs * 2 * TILE_T],
                            mybir.dt.float8e4, kind="ExternalInput")
             for i in range(NT)] if cs else []
    rt_ds = [nc.dram_tensor(f"rt{i}", [128, ct * 2 * S], mybir.dt.float8e4,
                            kind="ExternalInput") if ct else None
             for i, ct in enumerate(ct_list)]
    b_ds = [nc.dram_tensor(f"b{i}", [128, ct * 2 * TILE_T],
                           mybir.dt.float8e4, kind="ExternalInput") if ct
            else None
            for i, ct in enumerate(ct_list)]
    bs_d = (nc.dram_tensor("bs", [128, cs * 2 * S], mybir.dt.float8e4,
                           kind="ExternalInput") if cs else None)
    d_d = nc.dram_tensor("d", [128, NT * S], mybir.dt.float16,
                         kind="ExternalOutput")

    with TileContext(nc) as tc:
        with (
            tc.tile_pool(name="rsp", bufs=3) as rs_p,
            tc.tile_pool(name="rtp", bufs=3) as rt_p,
            tc.tile_pool(name="bp", bufs=3) as b_p,
            tc.tile_pool(name="bsp", bufs=1) as bs_p,
            tc.tile_pool(name="psp", bufs=4, space=bass.MemorySpace.PSUM) as ps_p,
            tc.tile_pool(name="dlo", bufs=NT) as dlo_p,
            tc.tile_pool(name="wup", bufs=1) as wu_p,
            tc.tile_pool(name="wups", bufs=1,
                         space=bass.MemorySpace.PSUM) as wups_p,
        ):
            # HAM warmup: keep PE busy through the DMA pipeline-fill head
            # so the real matmul stream starts at the warm 2.4GHz clock
            wu_w = wu_p.tile([128, TILE_T], mybir.dt.float8e4)
            wu_x = wu_p.tile([128, S], mybir.dt.bfloat16)
            wu_x8 = wu_p.tile([1, 4], mybir.dt.float8e4)
            nc.gpsimd.memset(wu_w[:], 0)
            nc.gpsimd.memset(wu_x[:], 0)
            nc.gpsimd.memset(wu_x8[:], 0)
            wu_ps = wups_p.tile([128, S], mybir.dt.float32)
            for _ in range(30):
                nc.tensor.matmul(wu_ps[:], wu_w[:], wu_x[:],
                                 start=True, stop=True)

            if cs:
                bs_sb = bs_p.tile([128, 2 * cs, S], mybir.dt.float8e4)
                nc.sync.dma_start(bs_sb[:, :, :], bs_d[:])

            for tt, ct in enumerate(ct_list):
                if ct:
                    rt_sb = rt_p.tile([128, 2 * ct, S], mybir.dt.float8e4)
                    nc.sync.dma_start(rt_sb[:, :, :], rt_ds[tt][:])
                    b_sb = b_p.tile([128, 2 * ct, TILE_T],
                                    mybir.dt.float8e4)
                    nc.sync.dma_start(b_sb[:, :, :], b_ds[tt][:])
                if cs:
                    rs_sb = rs_p.tile([128, 2 * cs, TILE_T],
                                      mybir.dt.float8e4)
                    nc.sync.dma_start(rs_sb[:, :, :], rs_ds[tt][:])
                ps = ps_p.tile([128, S], mybir.dt.float32)
                # wait-laundering: a 1x1 dummy matmul absorbs the psum
                # WAR wait (vs the drain of this bank's previous tile) on
                # the PE stream, so the first real matmul keeps only its
                # DMA RAW wait (walrus allows one wait per MM).
                nc.tensor.matmul(ps[0:1, 0:1], wu_w[0:1, 0:1],
                                 wu_x8[0:1, 0:1], start=False, stop=False,
                                 skip_group_check=True)
                npass = cs + ct
                idx = 0
                # tgt passes first: their DMAs (rt+b) are issued first
                for i in range(ct):
                    nc.tensor.matmul(
                        ps[:],
                        b_sb[:, 2 * i:2 * i + 2, :],
                        rt_sb[:, 2 * i:2 * i + 2, :],
                        start=(idx == 0), stop=(idx == npass - 1),
                        perf_mode=dr_mode,
                    )
                    idx += 1
                for i in range(cs):
                    nc.tensor.matmul(
                        ps[:],
                        rs_sb[:, 2 * i:2 * i + 2, :],
                        bs_sb[:, 2 * i:2 * i + 2, :],
                        start=(idx == 0), stop=(idx == npass - 1),
                        perf_mode=dr_mode,
                    )
                    idx += 1
                # PSUM drain alternates DVE/ACT per tile to spread work
                d_lo = dlo_p.tile([128, S], mybir.dt.float16)
                if tt % 2 == 0:
                    nc.vector.tensor_copy(d_lo[:], ps[:])
                else:
                    nc.scalar.copy(d_lo[:], ps[:])
                slot = out_slots[tt]
                nc.sync.dma_start(d_d[:, slot * S:(slot + 1) * S], d_lo[:])
    _strip_waw_waits(nc)
    return nc


_ENGINE_SEM_PREFIX = {
    "PE": "PE_",
    "DVE": "DVE_",
    "Activation": "Activation_",
    "SP": "SP_",
    "Pool": "Pool_",
}


def _strip_waw_waits(nc):
    """Reduce embedded sem waits to what walrus accepts (one per
    instruction for DMA/DVE/ACT). Two provably-redundant classes are
    dropped:
      - same-engine waits: engines execute their stream in order, so a
        wait on the instruction's own engine semaphore is already
        satisfied by program order;
      - DMA-completion (WAW) waits on reuse DMAs that also carry the
        consumer-engine WAR wait: the consumer's read of the old contents
        already waited on the old DMA's completion."""
    for inst in nc.all_instructions():
        si = getattr(inst, "sync_info", None)
        if not si or not si.on_wait or len(si.on_wait) <= 1:
            continue
        eng_name = getattr(getattr(inst, "engine", None), "name", "")
        own = _ENGINE_SEM_PREFIX.get(eng_name)
        waits = list(si.on_wait)
        if own is not None:
            waits = [w for w in waits if not (w.ant_name or "").startswith(own)]
        if type(inst).__name__ == "InstDMACopy" and len(waits) > 1:
            eng = [w for w in waits if "DMA" not in (w.ant_name or "")]
            assert len(eng) <= 1, (
                f"unexpected DMA wait set on {inst.name}: "
                f"{[w.ant_name for w in si.on_wait]}"
            )
            waits = eng
        si.on_wait = waits


def _get_nc(cs, ct_list, out_slots):
    key = ("nc", cs, tuple(ct_list), tuple(out_slots))
    if key not in _CACHE:
        _CACHE[key] = _build_nc(cs, ct_list, out_slots)
    return _CACHE[key]


def _idx_concat(A, F, C, E, lo, hi, n_desc):
    return np.concatenate([
        A[lo:hi].reshape(n_desc, 35),
        F[lo:hi].reshape(n_desc, 10),
        C[lo:hi].reshape(n_desc, 1),
        E[lo:hi].reshape(n_desc, 1),
    ], axis=1).astype(np.int64)                  # [n_desc, 47]


def _dr_layout(arr, npass, X):
    """[npass*256, X] row-major -> [128 part, npass*2*X] with DR plane
    pairing: logical row 256*i + 128*j + p  ->  partition p, block 2i+j."""
    return np.ascontiguousarray(
        arr.reshape(npass, 2, 128, X).transpose(2, 0, 1, 3)
    ).reshape(128, -1)


def _host_prep(q8u8, idxS, idxT_cores):
    """Build all device operands. Returns (cs, ct_list, orders, sizes,
    bs_map, per-core {rs,rt,b} arrays)."""
    f8 = ml_dtypes.float8_e4m3
    q8f = (q8u8.astype(np.float32) - SHIFT).astype(f8)   # [256,256] fp8

    # ---- k split: src-side slots = NS_K slots with fewest unique idxS
    uniq_s = [np.unique(idxS[:, k]) for k in range(K)]
    order_k = sorted(range(K), key=lambda k: len(uniq_s[k]))
    src_k = sorted(order_k[:NS_K])
    tgt_k = sorted(order_k[NS_K:])

    # ---- src side (shared by all cores and tiles) ----
    cs = 0
    bs_map = None
    src_rows_k = []      # (k, sorted unique c array)
    if src_k:
        nrows = sum(len(uniq_s[k]) for k in src_k)
        cs = (nrows + DR - 1) // DR
        nrp = cs * DR
        bs = np.zeros((nrp, S), np.float32)
        r0 = 0
        for k in src_k:
            cvals = uniq_s[k]
            src_rows_k.append((k, cvals, r0))
            j = np.searchsorted(cvals, idxS[:, k])
            bs[r0 + j, np.arange(S)] += W_K[k]
            r0 += len(cvals)
        bs_map = _dr_layout(bs.astype(f8), cs, S)

    # ---- tgt side: per-(core, tile) compressed row sets ----
    karr = np.arange(K, dtype=np.int64)[None, :] * 256
    sizes = [96, 96] + [TILE_T] * (NT - 2)
    starts = np.concatenate([[0], np.cumsum(sizes)]).astype(int)
    assert starts[-1] == TLOC

    kt = np.array(tgt_k, dtype=np.int64)
    rows_ct, counts = [], []
    for idxT in idxT_cores:
        rows_t = []
        for tt in range(NT):
            sl = idxT[starts[tt]:starts[tt + 1]][:, kt]       # [n_t, |kt|]
            rows_t.append(np.unique((kt[None, :] * 256 + sl).ravel())
                          if len(kt) else np.zeros(0, np.int64))
        rows_ct.append(rows_t)
        counts.append([(len(r) + DR - 1) // DR for r in rows_t])

    # per-core ascending sort of tiles by tgt chunk count: aligning the
    # order statistics tightens the position-wise maxima the SPMD
    # program pads to, and keeps the smallest tiles first
    orders = [sorted(range(NT), key=lambda tt: counts[c][tt])
              for c in range(NCORES)]
    ct_list = [
        max(counts[c][orders[c][i]] for c in range(NCORES))
        for i in range(NT)
    ]

    per_core = []
    q8fT = np.ascontiguousarray(q8f.T)  # for src-side row gathers
    for c, idxT in enumerate(idxT_cores):
        m = {}
        for i in range(NT):
            tt = orders[c][i]
            n_t = sizes[tt]
            sl_full = idxT[starts[tt]:starts[tt + 1]]          # [n_t, 47]
            # --- tgt side ---
            ct = ct_list[i]
            if ct:
                nrp = ct * DR
                rows = rows_ct[c][tt]
                nr = len(rows)
                rk = rows >> 8
                rc = rows & 255
                rt = np.zeros((nrp, S), f8)
                rt[:nr] = q8f[idxS[:, rk], rc[None, :]].T
                sl = sl_full[:, kt]
                pair = (kt[None, :] * 256 + sl)                # [n_t,|kt|]
                j = np.searchsorted(rows, pair.ravel())
                tcol = np.repeat(np.arange(n_t), len(kt))
                bm = np.zeros((nrp, TILE_T), np.float32)
                bm[j, tcol] += np.tile(W_K[kt], n_t)
                m[f"rt{i}"] = _dr_layout(rt, ct, S)
                m[f"b{i}"] = _dr_layout(bm.astype(f8), ct, TILE_T)
            # --- src side: Rs[(k,c), t] = q8[c, idxT[t,k]] - 128 ---
            if cs:
                nrp = cs * DR
                rs = np.zeros((nrp, TILE_T), f8)
                for k, cvals, r0 in src_rows_k:
                    # q8fT[j, c] = q8f[c, j]; gather rows by idxT[t, k]
                    rs[r0:r0 + len(cvals), :n_t] = \
                        q8fT[sl_full[:, k]][:, cvals].T
                m[f"rs{i}"] = _dr_layout(rs, cs, TILE_T)
        if cs:
            m["bs"] = bs_map
        per_core.append(m)
    return cs, ct_list, orders, sizes, per_core


def _reduce(D_full, align_10):
    """D_full: [128 m, 10 tc, 10 ta, 4 n, 10 sc, 10 sa] -> out [4, 128]."""
    cost = D_full.transpose(3, 0, 1, 4, 2, 5)    # [n,m,tc,sc,ta,sa]
    al = align_10[:, :10]                        # [60, 10]
    aligned = cost[..., al, np.arange(10)]       # [n,m,tc,sc,60,10]
    sum_diag = aligned.sum(-1)                   # [n,m,tc,sc,60]
    return sum_diag.reshape(N_SRC, M_TGT, -1).min(-1).astype(np.float32)


def kernel(q8_table, align_10,
           src_ArtCoeff, src_FdCoeff_q8, src_CirCoeff_q8, src_EccCoeff_q8,
           tgt_ArtCoeff, tgt_FdCoeff_q8, tgt_CirCoeff_q8, tgt_EccCoeff_q8,
           _trace=False):
    from concourse.bass_utils import run_bass_kernel_spmd

    q8u8 = np.asarray(q8_table).astype(np.uint8)
    idxS = _idx_concat(np.asarray(src_ArtCoeff), np.asarray(src_FdCoeff_q8),
                       np.asarray(src_CirCoeff_q8), np.asarray(src_EccCoeff_q8),
                       0, N_SRC, S)
    tA = np.asarray(tgt_ArtCoeff)
    tF = np.asarray(tgt_FdCoeff_q8)
    tC = np.asarray(tgt_CirCoeff_q8)
    tE = np.asarray(tgt_EccCoeff_q8)
    idxT_cores = [
        _idx_concat(tA, tF, tC, tE, i * MLOC, (i + 1) * MLOC, TLOC)
        for i in range(NCORES)
    ]
    cs, ct_list, orders, sizes, per_core = _host_prep(
        q8u8, idxS, idxT_cores)

    nc = _get_nc(cs, ct_list, list(range(NT)))
    res = run_bass_kernel_spmd(nc, per_core, core_ids=list(range(NCORES)),
                               trace=_trace)
    _CACHE["last_result"] = res
    _CACHE["total_ns"] = res.exec_time_ns if _trace else None

    # gather: per core, position i holds that core's orders[c][i]-th
    # tile; scatter each back to its t-range
    starts = np.concatenate([[0], np.cumsum(sizes)]).astype(int)
    D_parts = []
    for c in range(NCORES):
        d = np.asarray(res.results[c]["d"], np.float32).reshape(128, NT, S)
        d = d.transpose(1, 0, 2)                 # [NT(pos), 128, S]
        dt = np.empty((TLOC, S), np.float32)
        for i in range(NT):
            tt = orders[c][i]
            dt[starts[tt]:starts[tt + 1]] = d[i, :sizes[tt]]
        D_parts.append(dt.reshape(MLOC, 10, 10, N_SRC, 10, 10))
    D_full = np.concatenate(D_parts, axis=0) + BASE   # [128,10,10,4,10,10]
    return _reduce(D_full, np.asarray(align_10))


# revision 6
# speedup vs baseline: 1.6131x; 1.6131x over previous
"""LFD all-pairs distance kernel for 8 Trainium2 NeuronCores.

Strategy (data-parallel over tgt batch axis m, per sharding hint):
  - Each of the 8 cores owns 16 of the 128 tgt rows (1600 tgt descriptors).
  - The pairwise cost D[t, s] = sum_k w_k * q8_table[idxS[s,k], idxT[t,k]]
    (s = 400 src descriptors, t = 1600 tgt descriptors per core,
     k = 47 coefficient slots: 35 art + 10 fd(w=2) + cir(w=2) + ecc)
    is computed as fp8 one-hot contractions on TensorE with DoubleRow
    (256-row contraction per pass), accumulating in one PSUM tile per
    128-target output tile.  Two complementary factorizations share the
    accumulation; the k-slots are split between them to balance DMA
    bytes against PE passes:
      * tgt-side (per-tile row compression):
            D += B^T @ Rt,  Rt[(k,c), s] = q8[idxS[s,k], c] - 128,
            B[(k,c), t] = w_k if idxT[t,k] == c
        rows = per-tile unique (k, idxT) pairs; costs ~528B DMA/row-use.
      * src-side (row set fixed by the src indices, shared by all tiles
        AND all cores):
            D += Rs^T(t-part) with Bs streamed:
            Rs[(k,c), t] = q8[c, idxT[t,k]] - 128  (stationary, per tile)
            Bs[(k,c), s] = w_k if idxS[s,k] == c   (resident, loaded once)
        rows = global unique (k, idxS) pairs; costs ~128B DMA/row-use.
  - Everything ships as fp8_e4m3 of (table value - 128); |x| <= 128 so no
    TRN e4m3 clipping at 240, rounding err <= 4/el washes out over the
    47-term sums (measured end-to-end rel err ~2e-3 vs 2e-2 gate).  The
    constant 128*sum(w_k) = 7424 is added back on host.
  - Host does index re-encoding (one-hot/gather layouts, gathers of the
    pre-converted fp8 table bytes) + final alignment min-reduction.
"""

import numpy as np
import ml_dtypes

N_SRC = 4
M_TGT = 128
NCORES = 8
MLOC = M_TGT // NCORES      # 16 tgt rows per core
S = N_SRC * 100             # 400 src descriptors
TLOC = MLOC * 100           # 1600 tgt descriptors per core
TILE_T = 128
NT = (TLOC + TILE_T - 1) // TILE_T   # 13 t tiles
K = 47                      # coefficient slots
W_K = np.array([1.0] * 35 + [2.0] * 10 + [2.0, 1.0], np.float32)
SHIFT = 128.0
BASE = SHIFT * float(W_K.sum())      # 7424: added back on host
NS_K = 36                   # number of k-slots handled by the src-side
BS_CHUNK = 6                # src passes per bs-resident DMA chunk
N_WARM = 60                 # HAM warmup matmuls (cover DMA pipeline fill)
DR = 256                    # DoubleRow contraction rows per pass

_CACHE = {}


def _install_tile_patch():
    import concourse.mybir as mybir
    from concourse import tile as _tile_mod
    from concourse.vector_clock import ScopedClock as _ScopedClock

    if getattr(_tile_mod.TileContext, "_drain_split_patched", False):
        return

    def _drain_and_barrier(self, tick_clock, wait_clock):
        # walrus's setupSyncWait rejects instructions with many embedded
        # waits; spread the exit-drain's wait set over extra SP nops.
        drain_inst = self.nc.sync.drain()
        wait_clock.add_sem_waits(
            drain_inst.ins,
            _ScopedClock({None: tick_clock.global_clock}))
        si = drain_inst.ins.sync_info
        waits = list(si.on_wait or [])
        if len(waits) > 1:
            si.on_wait = waits[:1]
            for j in range(1, len(waits)):
                nop = self.nc.sync.nop()
                nop.ins.sync_info = mybir.SyncInfo(
                    on_wait=[waits[j]], on_update=[])
        self.nc.all_engine_barrier()
        assert self.sems is not None
        popped = self.nc._tile_sem_poison_stack.pop()
        assert popped is self._sem_poison
        self.nc.clear_and_free_semaphores(
            list(self.sems.allocated().values()))
        self.nc.all_engine_barrier()

    _tile_mod.TileContext._drain_and_barrier = _drain_and_barrier
    _tile_mod.TileContext._drain_split_patched = True


def _build_nc(cs, ct_list, out_slots):
    """cs: src-side DoubleRow pass count (same for every tile/core);
    ct_list: tgt-side DR pass count per tile position; out_slots: output
    slot per position."""
    import concourse.bass as bass
    import concourse.mybir as mybir
    from concourse.tile import TileContext

    _install_tile_patch()

    nc = bass.Bass()
    dr_mode = mybir.MatmulPerfMode.DoubleRow

    rs_ds = [nc.dram_tensor(f"rs{i}", [128, cs * 2 * TILE_T],
                            mybir.dt.float8e4, kind="ExternalInput")
             for i in range(NT)] if cs else []
    rt_ds = [nc.dram_tensor(f"rt{i}", [128, ct * 2 * S], mybir.dt.float8e4,
                            kind="ExternalInput") if ct else None
             for i, ct in enumerate(ct_list)]
    b_ds = [nc.dram_tensor(f"b{i}", [128, ct * 2 * TILE_T],
                           mybir.dt.float8e4, kind="ExternalInput") if ct
            else None
            for i, ct in enumerate(ct_list)]
    bs_d = (nc.dram_tensor("bs", [128, cs * 2 * S], mybir.dt.float8e4,
                           kind="ExternalInput") if cs else None)
    d_d = nc.dram_tensor("d", [128, NT * S], mybir.dt.float16,
                         kind="ExternalOutput")

    with TileContext(nc) as tc:
        with (
            tc.tile_pool(name="rsp", bufs=3) as rs_p,
            tc.tile_pool(name="rtp", bufs=3) as rt_p,
            tc.tile_pool(name="bp", bufs=3) as b_p,
            tc.tile_pool(name="bsp", bufs=1) as bs_p,
            tc.tile_pool(name="psp", bufs=4, space=bass.MemorySpace.PSUM) as ps_p,
            tc.tile_pool(name="dlo", bufs=NT) as dlo_p,
            tc.tile_pool(name="wup", bufs=1) as wu_p,
            tc.tile_pool(name="wups", bufs=1,
                         space=bass.MemorySpace.PSUM) as wups_p,
        ):
            # HAM warmup: keep PE busy through the DMA pipeline-fill head
            # so the real matmul stream starts at the warm 2.4GHz clock
            wu_w = wu_p.tile([128, TILE_T], mybir.dt.float8e4)
            wu_x = wu_p.tile([128, S], mybir.dt.bfloat16)
            wu_x8 = wu_p.tile([1, 4], mybir.dt.float8e4)
            nc.gpsimd.memset(wu_w[:], 0)
            nc.gpsimd.memset(wu_x[:], 0)
            nc.gpsimd.memset(wu_x8[:], 0)
            wu_ps = wups_p.tile([128, S], mybir.dt.float32)
            for _ in range(N_WARM):
                nc.tensor.matmul(wu_ps[:], wu_w[:], wu_x[:],
                                 start=True, stop=True)

            # DMA issue: rs + output ride the SP HWDGE ring; rt/b/bs ride
            # the ACT HWDGE ring — two descriptor rings dispatch in
            # parallel.  bs is chunked so early src passes of tile 0
            # don't wait for the whole resident tensor.
            bs_sb = None
            if cs:
                bs_sb = bs_p.tile([128, 2 * cs, S], mybir.dt.float8e4)

            for tt, ct in enumerate(ct_list):
                if ct:
                    rt_sb = rt_p.tile([128, 2 * ct, S], mybir.dt.float8e4)
                    nc.scalar.dma_start(rt_sb[:, :, :], rt_ds[tt][:])
                    b_sb = b_p.tile([128, 2 * ct, TILE_T],
                                    mybir.dt.float8e4)
                    nc.scalar.dma_start(b_sb[:, :, :], b_ds[tt][:])
                if cs:
                    rs_sb = rs_p.tile([128, 2 * cs, TILE_T],
                                      mybir.dt.float8e4)
                    nc.sync.dma_start(rs_sb[:, :, :], rs_ds[tt][:])
                if cs and tt == 0:
                    for c0 in range(0, cs, BS_CHUNK):
                        c1 = min(c0 + BS_CHUNK, cs)
                        nc.scalar.dma_start(
                            bs_sb[:, 2 * c0:2 * c1, :],
                            bs_d[:, 2 * c0 * S:2 * c1 * S])
                ps = ps_p.tile([128, S], mybir.dt.float32)
                # wait-laundering: a 1x1 dummy matmul absorbs the psum
                # WAR wait (vs the drain of this bank's previous tile) on
                # the PE stream, so the first real matmul keeps only its
                # DMA RAW wait (walrus allows one wait per MM).
                nc.tensor.matmul(ps[0:1, 0:1], wu_w[0:1, 0:1],
                                 wu_x8[0:1, 0:1], start=False, stop=False,
                                 skip_group_check=True)
                npass = cs + ct
                idx = 0
                # tgt passes first: their DMAs (rt+b) are issued first
                for i in range(ct):
                    nc.tensor.matmul(
                        ps[:],
                        b_sb[:, 2 * i:2 * i + 2, :],
                        rt_sb[:, 2 * i:2 * i + 2, :],
                        start=(idx == 0), stop=(idx == npass - 1),
                        perf_mode=dr_mode,
                    )
                    idx += 1
                for i in range(cs):
                    nc.tensor.matmul(
                        ps[:],
                        rs_sb[:, 2 * i:2 * i + 2, :],
                        bs_sb[:, 2 * i:2 * i + 2, :],
                        start=(idx == 0), stop=(idx == npass - 1),
                        perf_mode=dr_mode,
                    )
                    idx += 1
                # PSUM drain alternates DVE/ACT per tile to spread work
                d_lo = dlo_p.tile([128, S], mybir.dt.float16)
                if tt % 2 == 0:
                    nc.vector.tensor_copy(d_lo[:], ps[:])
                else:
                    nc.scalar.copy(d_lo[:], ps[:])
                slot = out_slots[tt]
                nc.sync.dma_start(d_d[:, slot * S:(slot + 1) * S], d_lo[:])
    _strip_waw_waits(nc)
    return nc


_ENGINE_SEM_PREFIX = {
    "PE": "PE_",
    "DVE": "DVE_",
    "Activation": "Activation_",
    "SP": "SP_",
    "Pool": "Pool_",
}


def _strip_waw_waits(nc):
    """Reduce embedded sem waits to what walrus accepts (one per
    instruction for DMA/DVE/ACT). Two provably-redundant classes are
    dropped:
      - same-engine waits: engines execute their stream in order, so a
        wait on the instruction's own engine semaphore is already
        satisfied by program order;
      - DMA-completion (WAW) waits on reuse DMAs that also carry the
        consumer-engine WAR wait: the consumer's read of the old contents
        already waited on the old DMA's completion."""
    for inst in nc.all_instructions():
        si = getattr(inst, "sync_info", None)
        if not si or not si.on_wait or len(si.on_wait) <= 1:
            continue
        eng_name = getattr(getattr(inst, "engine", None), "name", "")
        own = _ENGINE_SEM_PREFIX.get(eng_name)
        waits = list(si.on_wait)
        if own is not None:
            waits = [w for w in waits if not (w.ant_name or "").startswith(own)]
        if type(inst).__name__ == "InstDMACopy" and len(waits) > 1:
            eng = [w for w in waits if "DMA" not in (w.ant_name or "")]
            assert len(eng) <= 1, (
                f"unexpected DMA wait set on {inst.name}: "
                f"{[w.ant_name for w in si.on_wait]}"
            )
            waits = eng
        si.on_wait = waits


def _get_nc(cs, ct_list, out_slots):
    key = ("nc", cs, tuple(ct_list), tuple(out_slots))
    if key not in _CACHE:
        _CACHE[key] = _build_nc(cs, ct_list, out_slots)
    return _CACHE[key]


def _idx_concat(A, F, C, E, lo, hi, n_desc):
    return np.concatenate([
        A[lo:hi].reshape(n_desc, 35),
        F[lo:hi].reshape(n_desc, 10),
        C[lo:hi].reshape(n_desc, 1),
        E[lo:hi].reshape(n_desc, 1),
    ], axis=1).astype(np.int64)                  # [n_desc, 47]


def _dr_layout(arr, npass, X):
    """[npass*256, X] row-major -> [128 part, npass*2*X] with DR plane
    pairing: logical row 256*i + 128*j + p  ->  partition p, block 2i+j."""
    return np.ascontiguousarray(
        arr.reshape(npass, 2, 128, X).transpose(2, 0, 1, 3)
    ).reshape(128, -1)


def _host_prep(q8u8, idxS, idxT_cores):
    """Build all device operands. Returns (cs, ct_list, orders, sizes,
    bs_map, per-core {rs,rt,b} arrays)."""
    f8 = ml_dtypes.float8_e4m3
    q8f = (q8u8.astype(np.float32) - SHIFT).astype(f8)   # [256,256] fp8

    # ---- k split: src-side slots = NS_K slots with fewest unique idxS
    uniq_s = [np.unique(idxS[:, k]) for k in range(K)]
    order_k = sorted(range(K), key=lambda k: len(uniq_s[k]))
    src_k = sorted(order_k[:NS_K])
    tgt_k = sorted(order_k[NS_K:])

    # ---- src side (shared by all cores and tiles) ----
    cs = 0
    bs_map = None
    src_rows_k = []      # (k, sorted unique c array)
    if src_k:
        nrows = sum(len(uniq_s[k]) for k in src_k)
        cs = (nrows + DR - 1) // DR
        nrp = cs * DR
        bs = np.zeros((nrp, S), np.float32)
        r0 = 0
        for k in src_k:
            cvals = uniq_s[k]
            src_rows_k.append((k, cvals, r0))
            j = np.searchsorted(cvals, idxS[:, k])
            bs[r0 + j, np.arange(S)] += W_K[k]
            r0 += len(cvals)
        bs_map = _dr_layout(bs.astype(f8), cs, S)

    # ---- tgt side: per-(core, tile) compressed row sets ----
    karr = np.arange(K, dtype=np.int64)[None, :] * 256
    sizes = [96, 96] + [TILE_T] * (NT - 2)
    starts = np.concatenate([[0], np.cumsum(sizes)]).astype(int)
    assert starts[-1] == TLOC

    kt = np.array(tgt_k, dtype=np.int64)
    rows_ct, counts = [], []
    for idxT in idxT_cores:
        rows_t = []
        for tt in range(NT):
            sl = idxT[starts[tt]:starts[tt + 1]][:, kt]       # [n_t, |kt|]
            rows_t.append(np.unique((kt[None, :] * 256 + sl).ravel())
                          if len(kt) else np.zeros(0, np.int64))
        rows_ct.append(rows_t)
        counts.append([(len(r) + DR - 1) // DR for r in rows_t])

    # per-core ascending sort of tiles by tgt chunk count: aligning the
    # order statistics tightens the position-wise maxima the SPMD
    # program pads to, and keeps the smallest tiles first
    orders = [sorted(range(NT), key=lambda tt: counts[c][tt])
              for c in range(NCORES)]
    ct_list = [
        max(counts[c][orders[c][i]] for c in range(NCORES))
        for i in range(NT)
    ]

    per_core = []
    q8fT = np.ascontiguousarray(q8f.T)  # for src-side row gathers
    for c, idxT in enumerate(idxT_cores):
        m = {}
        for i in range(NT):
            tt = orders[c][i]
            n_t = sizes[tt]
            sl_full = idxT[starts[tt]:starts[tt + 1]]          # [n_t, 47]
            # --- tgt side ---
            ct = ct_list[i]
            if ct:
                nrp = ct * DR
                rows = rows_ct[c][tt]
                nr = len(rows)
                rk = rows >> 8
                rc = rows & 255
                rt = np.zeros((nrp, S), f8)
                rt[:nr] = q8f[idxS[:, rk], rc[None, :]].T
                sl = sl_full[:, kt]
                pair = (kt[None, :] * 256 + sl)                # [n_t,|kt|]
                j = np.searchsorted(rows, pair.ravel())
                tcol = np.repeat(np.arange(n_t), len(kt))
                bm = np.zeros((nrp, TILE_T), np.float32)
                bm[j, tcol] += np.tile(W_K[kt], n_t)
                m[f"rt{i}"] = _dr_layout(rt, ct, S)
                m[f"b{i}"] = _dr_layout(bm.astype(f8), ct, TILE_T)
            # --- src side: Rs[(k,c), t] = q8[c, idxT[t,k]] - 128 ---
            if cs:
                nrp = cs * DR
                rs = np.zeros((nrp, TILE_T), f8)
                for k, cvals, r0 in src_rows_k:
                    # q8fT[j, c] = q8f[c, j]; gather rows by idxT[t, k]
                    rs[r0:r0 + len(cvals), :n_t] = \
                        q8fT[sl_full[:, k]][:, cvals].T
                m[f"rs{i}"] = _dr_layout(rs, cs, TILE_T)
        if cs:
            m["bs"] = bs_map
        per_core.append(m)
    return cs, ct_list, orders, sizes, per_core


def _reduce(D_full, align_10):
    """D_full: [128 m, 10 tc, 10 ta, 4 n, 10 sc, 10 sa] -> out [4, 128]."""
    cost = D_full.transpose(3, 0, 1, 4, 2, 5)    # [n,m,tc,sc,ta,sa]
    al = align_10[:, :10]                        # [60, 10]
    aligned = cost[..., al, np.arange(10)]       # [n,m,tc,sc,60,10]
    sum_diag = aligned.sum(-1)                   # [n,m,tc,sc,60]
    return sum_diag.reshape(N_SRC, M_TGT, -1).min(-1).astype(np.float32)


def kernel(q8_table, align_10,
           src_ArtCoeff, src_FdCoeff_q8, src_CirCoeff_q8, src_EccCoeff_q8,
           tgt_ArtCoeff, tgt_FdCoeff_q8, tgt_CirCoeff_q8, tgt_EccCoeff_q8,
           _trace=False):
    from concourse.bass_utils import run_bass_kernel_spmd

    q8u8 = np.asarray(q8_table).astype(np.uint8)
    idxS = _idx_concat(np.asarray(src_ArtCoeff), np.asarray(src_FdCoeff_q8),
                       np.asarray(src_CirCoeff_q8), np.asarray(src_EccCoeff_q8),
                       0, N_SRC, S)
    tA = np.asarray(tgt_ArtCoeff)
    tF = np.asarray(tgt_FdCoeff_q8)
    tC = np.asarray(tgt_CirCoeff_q8)
    tE = np.asarray(tgt_EccCoeff_q8)
    idxT_cores = [
        _idx_concat(tA, tF, tC, tE, i * MLOC, (i + 1) * MLOC, TLOC)
        for i in range(NCORES)
    ]
    cs, ct_list, orders, sizes, per_core = _host_prep(
        q8u8, idxS, idxT_cores)

    nc = _get_nc(cs, ct_list, list(range(NT)))
    res = run_bass_kernel_spmd(nc, per_core, core_ids=list(range(NCORES)),
                               trace=_trace)
    _CACHE["last_result"] = res
    _CACHE["total_ns"] = res.exec_time_ns if _trace else None

    # gather: per core, position i holds that core's orders[c][i]-th
    # tile; scatter each back to its t-range
    starts = np.concatenate([[0], np.cumsum(sizes)]).astype(int)
    D_parts = []
    for c in range(NCORES):
        d = np.asarray(res.results[c]["d"], np.float32).reshape(128, NT, S)
        d = d.transpose(1, 0, 2)                 # [NT(pos), 128, S]
        dt = np.empty((TLOC, S), np.float32)
        for i in range(NT):
            tt = orders[c][i]
            dt[starts[tt]:starts[tt + 1]] = d[i, :sizes[tt]]
        D_parts.append(dt.reshape(MLOC, 10, 10, N_SRC, 10, 10))
    D_full = np.concatenate(D_parts, axis=0) + BASE   # [128,10,10,4,10,10]
    return _reduce(D_full, np.asarray(align_10))
